# revision 14
# baseline (speedup 1.0000x reference)
"""Trainium2 Bass kernel for BidPrefix, v5: fp16 2x_1P paged masked sums.

Reference semantics (per row b of inputs [B, 302]):
  rates = inputs[b, :300]; bid = int(inputs[b, 300]); mp = int(inputs[b, 301])
  cpz[k] = prod(rates[:k]) (cpz[0] = 1); out[b] = [cpz[bid], cpz[mp+1], cpz[mp]]

Log-space masked prefix sums (cpz[idx] = exp(sum_{j<idx} ln r_j)) with a
hand-authored 2-elements/cycle (2x_1P perf mode) fp16 custom DVE op,
validated on HW by probe2x.py:
  - ScalarE: la = ln(rates) -> fp16 [128, T, 300]
  - per output i (th = bid / mp+1 / mp): src1 = full-rate fp16 pair stream,
    page t = (A, B) = (th/2, (th-1)/2) repeated 150x. Built by seeding 8
    pairs with strided tensor_scalar writes, then 5 log-doubling SBUF->SBUF
    DMA copies over a flat [128, 3*Tm pages, k] layout (one DMA covers all
    3 streams).
  - custom op (3 uops: entry-reset / steady / reset at SUB_DIM_DONE): per
    cycle processes a packed fp16 pair: two pair-counter scan states c,
    m_lo = c < A, m_hi = c < B, S += m_lo*x_lo + m_hi*x_hi; counters and S
    reset each page, so the page-end out element is the per-row masked sum.
  - page-end sums -> one Exp pass (ScalarE) -> out.
"""

import sys

if "/opt/trn_rl_repo" not in sys.path:
    sys.path.insert(0, "/opt/trn_rl_repo")

import numpy as np

S = 300
COLS = 302
P = 128
NCORES = 8
TILES = 196
GROUP = 24
BPC = TILES * P
BTOT = 200000

TRACE = False
LAST_RESULTS = None

_OP = None
_HAND = {}


def _build_programs():
    from concourse.dve_uop import (
        AluInp, AluOp, DelayInp, InpSel, OutPath, OutSel, Trigger,
        UopConfig, UopDpConfig,
    )

    ENABLE = 1

    def mk_uop(stages, lanes_used, captures, out_both, trigger, next_uop,
               repeat, consume):
        u = UopConfig()
        dp = []
        for b in range(8):
            blk = UopDpConfig()
            blk.pass_through_delay(*lanes_used)
            if b in stages:
                op, s0, s1 = stages[b]
                blk.enable_alu(op, s0, s1)
            else:
                blk.pass_through_alu()
            if b in captures:
                blk.enable_delay_from_src(DelayInp.PREV_ALU_OUT, captures[b])
            dp.append(blk)
        u.datapath_config = dp
        u.out[OutPath.WR0_LO] = OutSel.ALU_OUT
        u.out_enable[OutPath.WR0_LO] = ENABLE
        if out_both:
            u.out[OutPath.WR0_HI] = OutSel.ALU_OUT
            u.out_enable[OutPath.WR0_HI] = ENABLE
        u.trigger = trigger
        u.next_uop = next_uop
        u.repeat_count = repeat
        u.require_inp0, u.require_inp1 = consume
        return u

    def fsm(mk_body):
        t_entry = (Trigger.SRC_TENSOR_DONE, Trigger.SUB_DIM_DONE, Trigger.COUNT)
        t_stdy = (Trigger.SRC_TENSOR_DONE, Trigger.SUB_DIM_DONE, Trigger.NONE)
        return [
            mk_body(True, t_entry, (0, 2, 1), 1),
            mk_body(False, t_stdy, (0, 2, 0), 0),
            mk_body(True, t_entry, (0, 2, 1), 1),
        ]

    PREV = AluInp.PREV_ALU_OUT
    CURR = AluInp.CURR_ALU_OUT
    D = [AluInp.PREV_DELAY_0, AluInp.PREV_DELAY_1, AluInp.PREV_DELAY_2,
         AluInp.PREV_DELAY_3, AluInp.PREV_DELAY_4, AluInp.PREV_DELAY_5]

    # 1x base: half-step counter vs alternating (A, B) stream elements.
    # lanes: L0=SRC_0, L1=SRC_1, L2=CONST_0(=0.5)
    def mk_1x(reset, trigger, next_uop, repeat):
        stages = {
            0: (AluOp.SUBTRACT, CURR, CURR) if reset else (AluOp.ADD, CURR, D[2]),
            1: (AluOp.IS_LT, PREV, D[1]),
            2: (AluOp.MULTIPLY, PREV, D[0]),
            3: (AluOp.BYPASS, PREV, PREV) if reset else (AluOp.ADD, CURR, PREV),
        }
        u = mk_uop(stages, (0, 1, 2), {}, False, trigger, next_uop, repeat,
                   (1, 1))
        u.enable_input(InpSel.SRC_0, 1)
        u.enable_input(InpSel.SRC_1, 2)
        u.enable_input(InpSel.CONST_0, 3)
        return u

    # 2x_1P (probe-validated): pair counter c in two scan states;
    # m_lo = c < A (SRC_1), m_hi = c < B (SRC_1_HI).
    # lanes: L0=SRC_0, L1=SRC_0_HI, L2=SRC_1, L3=SRC_1_HI, L4=ONE, L5=capture
    def mk_2x(reset, trigger, next_uop, repeat):
        ctr = (AluOp.SUBTRACT, CURR, CURR) if reset else (AluOp.ADD, CURR, D[4])
        stages = {
            0: ctr,                                # c
            1: (AluOp.IS_LT, PREV, D[2]),          # m_lo = c < A
            2: (AluOp.MULTIPLY, PREV, D[0]),       # v_lo = m_lo * x_lo
            3: ctr,                                # c (2nd scan) + capture v_lo
            4: (AluOp.IS_LT, PREV, D[3]),          # m_hi = c < B
            5: (AluOp.MULTIPLY, PREV, D[1]),       # v_hi
            6: (AluOp.ADD, PREV, D[5]),            # pair = v_hi + v_lo
            7: (AluOp.BYPASS, PREV, PREV) if reset else (AluOp.ADD, CURR, PREV),
        }
        u = mk_uop(stages, (0, 1, 2, 3, 4, 5), {3: 5}, True,
                   trigger, next_uop, repeat, (1, 1))
        u.enable_input(InpSel.SRC_0, 1)
        u.enable_input(InpSel.SRC_0_HI, 2)
        u.enable_input(InpSel.SRC_1, 3)
        u.enable_input(InpSel.SRC_1_HI, 4)
        u.enable_input(InpSel.ONE_F32, 5)
        return u

    return fsm(mk_1x), fsm(mk_2x)


def _get_op():
    global _OP
    if _OP is not None:
        return _OP
    import concourse.dve_ops as dve_ops
    from concourse.dve_ops import OPS, DveOp
    from concourse.dve_spec import AluOp as SAluOp, Bin, Idx, Scan, Spec, Src0, Src1
    from concourse.dve_uop import DveOpSpec

    name = "MS2XU_ANT"
    for op in OPS:
        if op.name == name:
            _OP = op
            return op

    def _ref(in0, in1, s0, s1, imm2):
        x = in0.astype(np.float32)
        x = x.reshape(x.shape[0], -1, S)
        a = np.asarray(in1, np.float32).reshape(x.shape[0], -1, S)
        th = 2.0 * a[:, :, 0]
        mask = np.arange(S, dtype=np.float32)[None, None, :] < th[:, :, None]
        body = x * mask
        return np.cumsum(body, axis=2, dtype=np.float32).reshape(in0.shape[0], -1)

    masked = Bin(SAluOp.MULTIPLY, Bin(SAluOp.IS_LT, Idx, Src1), Src0)
    sc = object.__new__(Scan)
    object.__setattr__(sc, "op", SAluOp.ADD)
    object.__setattr__(sc, "expr", masked)
    object.__setattr__(sc, "init", None)
    object.__setattr__(sc, "_subdim_step", None)
    spec = Spec(body=sc, reference=_ref)

    class DveOpHand(DveOp):
        def compile(self, ver):
            return _HAND[(self.name, ver)]

    op = DveOpHand(name, spec, subdim=True, uops_sha={})
    OPS.append(op)
    row = dve_ops._CUSTOM_DVE_ROW_BASE + len(OPS) - 1
    dve_ops._SUB_OPCODE_FOR_NAME[name] = row
    dve_ops.CUSTOM_DVE_SPECS[name] = spec

    u1x, u2x = _build_programs()
    for ver in ("v3", "v4"):
        _HAND[(name, ver)] = DveOpSpec(
            name=name, uops=u1x, opcode=row, uops_2x=u2x, perf_max=1,
            rd1_en=True,
        )
    _OP = op
    return op


def _group_sizes(tiles, group):
    sizes = []
    rem = tiles
    for s_ in (2, 3, 5, 8, group // 2):
        if s_ >= 2 and rem - s_ >= group:
            sizes.append(s_)
            rem -= s_
    while rem > 0:
        s_ = min(group, rem)
        sizes.append(s_)
        rem -= s_
    return sizes


def build_nc(tiles=TILES, group=GROUP):
    import concourse.bacc as bacc
    import concourse.mybir as mybir
    from concourse import tile

    f32 = mybir.dt.float32
    f16 = mybir.dt.float16
    A_ = mybir.AluOpType
    AF = mybir.ActivationFunctionType
    OP = _get_op()

    if tiles < group:
        group = tiles
    sizes = _group_sizes(tiles, group)
    Tm = max(sizes)
    bpc = tiles * P
    SEED = 16 if S >= 32 else 2  # elements (8 pairs) seeded by DVE writes

    nc = bacc.Bacc("TRN2", target_bir_lowering=False, debug=False)
    inp = nc.dram_tensor("inp", [bpc, COLS], f32, kind="ExternalInput")
    out = nc.dram_tensor("out", [bpc, 3], f32, kind="ExternalOutput")

    vin = inp.ap().rearrange("(p t) c -> p t c", p=P)
    vout = out.ap().rearrange("(p t) k -> p t k", p=P)

    with tile.TileContext(nc) as tc:
        with (
            tc.tile_pool(name="raw", bufs=2) as rawp,
            tc.tile_pool(name="la", bufs=2) as lap,
            tc.tile_pool(name="idx", bufs=3) as idxp,
            tc.tile_pool(name="st", bufs=2) as stp,
            tc.tile_pool(name="jk", bufs=1) as jkp,
            tc.tile_pool(name="per", bufs=1) as perp,
        ):
            outlog = perp.tile([P, tiles, 3], f16)
            outf = perp.tile([P, tiles, 3], f32)
            junk = jkp.tile([P, Tm, S], f16)

            t0 = 0
            for T in sizes:
                rawf = rawp.tile([P, Tm, S], f32, tag="raw")
                raw = rawf[:, 0:T, :]
                nc.sync.dma_start(raw, vin[:, t0 : t0 + T, 0:S])
                icolsf = idxp.tile([P, Tm, 2], f32, tag="icols")
                if T < Tm:
                    nc.vector.memset(icolsf, 0.0)
                icols = icolsf[:, 0:T, :]
                nc.sync.dma_start(icols, vin[:, t0 : t0 + T, S:COLS])

                # seed 8 (A, B) pairs per page into the flat stream buffer:
                # stream i occupies pages [i*Tm, i*Tm+T); A at even offsets,
                # B = A - 0.5 at odd offsets. Seeds cover all Tm pages so the
                # doubling DMAs below never read uninitialized memory.
                stf = stp.tile([P, 3 * Tm, S], f16, tag="ast")
                nseed = SEED // 2
                bidb = icolsf[:, :, 0].unsqueeze(2).broadcast_to([P, Tm, nseed])
                mpb = icolsf[:, :, 1].unsqueeze(2).broadcast_to([P, Tm, nseed])
                for i, (srcb, mul, add) in enumerate((
                    (bidb, 0.5, None),      # A = bid/2
                    (mpb, 0.5, 0.5),        # A = (mp+1)/2
                    (mpb, 0.5, None),       # A = mp/2
                )):
                    pg = stf[:, i * Tm : (i + 1) * Tm, :]
                    evens = pg[:, :, 0 : SEED : 2]
                    odds = pg[:, :, 1 : SEED : 2]
                    if add is None:
                        nc.vector.tensor_scalar(evens, srcb, mul, None, A_.mult)
                        nc.vector.tensor_scalar(
                            odds, srcb, mul, -0.5, A_.mult, A_.add
                        )
                    else:
                        nc.vector.tensor_scalar(
                            evens, srcb, mul, add, A_.mult, A_.add
                        )
                        nc.vector.tensor_scalar(
                            odds, srcb, mul, add - 0.5, A_.mult, A_.add
                        )

                # log-doubling replicate across all 3*Tm pages at once
                k = SEED
                while k < S:
                    n = min(k, S - k)
                    nc.sync.dma_start(
                        stf[:, :, k : k + n], stf[:, :, 0:n]
                    )
                    k += n

                la = lap.tile([P, Tm, S], f16, tag="la")
                lag = la[:, 0:T, :]
                nc.scalar.activation(
                    lag.rearrange("p t s -> p (t s)"),
                    raw.rearrange("p t s -> p (t s)"),
                    AF.Ln,
                )

                for i in range(3):
                    jg = junk[:, 0:T, :]
                    bi = nc.vector._custom_dve(
                        OP,
                        out=jg,
                        in0=lag,
                        in1=stf[:, i * Tm : i * Tm + T, :],
                        s0=0.5,
                    )
                    # _custom_dve does not plumb perf_max; without it the
                    # engine never reaches the 2x_1P uop slot (byte-36[7:6]).
                    bi.ins.perf_max = 1
                    # page-end sums -> outlog[p, t, i]
                    nc.vector.tensor_copy(
                        outlog[:, t0 : t0 + T, i], jg[:, :, S - 1]
                    )
                t0 += T

            nc.scalar.activation(
                outf.rearrange("p t k -> p (t k)"),
                outlog.rearrange("p t k -> p (t k)"),
                AF.Exp,
            )
            nc.sync.dma_start(vout, outf)

    nc.compile()
    return nc


_NC_CACHE = {}


def _get_nc():
    key = (TILES, GROUP)
    if key not in _NC_CACHE:
        _NC_CACHE[key] = build_nc()
    return _NC_CACHE[key]


def kernel(inputs):
    global LAST_RESULTS
    x = np.ascontiguousarray(np.asarray(inputs), dtype=np.float32)
    assert x.shape == (BTOT, COLS), x.shape

    npad = BPC * NCORES - BTOT
    padrows = np.zeros((npad, COLS), dtype=np.float32)
    padrows[:, :S] = 1.0
    xp = np.concatenate([x, padrows], axis=0)
    shards = xp.reshape(NCORES, BPC, COLS)

    in_maps = [{"inp": np.ascontiguousarray(shards[c])} for c in range(NCORES)]

    nc = _get_nc()
    from concourse.bass_utils import run_bass_kernel_spmd

    r = run_bass_kernel_spmd(
        nc, in_maps, core_ids=list(range(NCORES)), trace=TRACE
    )
    LAST_RESULTS = r
    y = np.concatenate([r.results[c]["out"] for c in range(NCORES)], axis=0)
    return np.ascontiguousarray(y[:BTOT]).astype(np.float32)


# revision 16
# speedup vs baseline: 1.1502x; 1.1502x over previous
"""Trainium2 Bass kernel for BidPrefix, v5: fp16 2x_1P paged masked sums.

Reference semantics (per row b of inputs [B, 302]):
  rates = inputs[b, :300]; bid = int(inputs[b, 300]); mp = int(inputs[b, 301])
  cpz[k] = prod(rates[:k]) (cpz[0] = 1); out[b] = [cpz[bid], cpz[mp+1], cpz[mp]]

Log-space masked prefix sums (cpz[idx] = exp(sum_{j<idx} ln r_j)) with a
hand-authored 2-elements/cycle (2x_1P perf mode) fp16 custom DVE op,
validated on HW by probe2x.py:
  - ScalarE: la = ln(rates) -> fp16 [128, T, 300]
  - per output i (th = bid / mp+1 / mp): src1 = full-rate fp16 pair stream,
    page t = (A, B) = (th/2, (th-1)/2) repeated 150x. Built by seeding 8
    pairs with strided tensor_scalar writes, then 5 log-doubling SBUF->SBUF
    DMA copies over a flat [128, 3*Tm pages, k] layout (one DMA covers all
    3 streams).
  - custom op (3 uops: entry-reset / steady / reset at SUB_DIM_DONE): per
    cycle processes a packed fp16 pair: two pair-counter scan states c,
    m_lo = c < A, m_hi = c < B, S += m_lo*x_lo + m_hi*x_hi; counters and S
    reset each page, so the page-end out element is the per-row masked sum.
  - page-end sums -> one Exp pass (ScalarE) -> out.
"""

import sys

if "/opt/trn_rl_repo" not in sys.path:
    sys.path.insert(0, "/opt/trn_rl_repo")

import numpy as np

S = 300
COLS = 302
P = 128
NCORES = 8
TILES = 196
GROUP = 20
BPC = TILES * P
BTOT = 200000

TRACE = False
LAST_RESULTS = None

_OP = None
_HAND = {}


def _build_programs():
    from concourse.dve_uop import (
        AluInp, AluOp, DelayInp, InpSel, OutPath, OutSel, Trigger,
        UopConfig, UopDpConfig,
    )

    ENABLE = 1

    def mk_uop(stages, lanes_used, captures, out_both, trigger, next_uop,
               repeat, consume):
        u = UopConfig()
        dp = []
        for b in range(8):
            blk = UopDpConfig()
            blk.pass_through_delay(*lanes_used)
            if b in stages:
                op, s0, s1 = stages[b]
                blk.enable_alu(op, s0, s1)
            else:
                blk.pass_through_alu()
            if b in captures:
                blk.enable_delay_from_src(DelayInp.PREV_ALU_OUT, captures[b])
            dp.append(blk)
        u.datapath_config = dp
        u.out[OutPath.WR0_LO] = OutSel.ALU_OUT
        u.out_enable[OutPath.WR0_LO] = ENABLE
        if out_both:
            u.out[OutPath.WR0_HI] = OutSel.ALU_OUT
            u.out_enable[OutPath.WR0_HI] = ENABLE
        u.trigger = trigger
        u.next_uop = next_uop
        u.repeat_count = repeat
        u.require_inp0, u.require_inp1 = consume
        return u

    def fsm(mk_body):
        t_entry = (Trigger.SRC_TENSOR_DONE, Trigger.SUB_DIM_DONE, Trigger.COUNT)
        t_stdy = (Trigger.SRC_TENSOR_DONE, Trigger.SUB_DIM_DONE, Trigger.NONE)
        return [
            mk_body(True, t_entry, (0, 2, 1), 1),
            mk_body(False, t_stdy, (0, 2, 0), 0),
            mk_body(True, t_entry, (0, 2, 1), 1),
        ]

    PREV = AluInp.PREV_ALU_OUT
    CURR = AluInp.CURR_ALU_OUT
    D = [AluInp.PREV_DELAY_0, AluInp.PREV_DELAY_1, AluInp.PREV_DELAY_2,
         AluInp.PREV_DELAY_3, AluInp.PREV_DELAY_4, AluInp.PREV_DELAY_5]

    # 1x base: half-step counter vs alternating (A, B) stream elements.
    # lanes: L0=SRC_0, L1=SRC_1, L2=CONST_0(=0.5)
    def mk_1x(reset, trigger, next_uop, repeat):
        stages = {
            0: (AluOp.SUBTRACT, CURR, CURR) if reset else (AluOp.ADD, CURR, D[2]),
            1: (AluOp.IS_LT, PREV, D[1]),
            2: (AluOp.MULTIPLY, PREV, D[0]),
            3: (AluOp.BYPASS, PREV, PREV) if reset else (AluOp.ADD, CURR, PREV),
        }
        u = mk_uop(stages, (0, 1, 2), {}, False, trigger, next_uop, repeat,
                   (1, 1))
        u.enable_input(InpSel.SRC_0, 1)
        u.enable_input(InpSel.SRC_1, 2)
        u.enable_input(InpSel.CONST_0, 3)
        return u

    # 2x_1P (probe-validated): pair counter c in two scan states;
    # m_lo = c < A (SRC_1), m_hi = c < B (SRC_1_HI).
    # lanes: L0=SRC_0, L1=SRC_0_HI, L2=SRC_1, L3=SRC_1_HI, L4=ONE, L5=capture
    def mk_2x(reset, trigger, next_uop, repeat):
        ctr = (AluOp.SUBTRACT, CURR, CURR) if reset else (AluOp.ADD, CURR, D[4])
        stages = {
            0: ctr,                                # c
            1: (AluOp.IS_LT, PREV, D[2]),          # m_lo = c < A
            2: (AluOp.MULTIPLY, PREV, D[0]),       # v_lo = m_lo * x_lo
            3: ctr,                                # c (2nd scan) + capture v_lo
            4: (AluOp.IS_LT, PREV, D[3]),          # m_hi = c < B
            5: (AluOp.MULTIPLY, PREV, D[1]),       # v_hi
            6: (AluOp.ADD, PREV, D[5]),            # pair = v_hi + v_lo
            7: (AluOp.BYPASS, PREV, PREV) if reset else (AluOp.ADD, CURR, PREV),
        }
        u = mk_uop(stages, (0, 1, 2, 3, 4, 5), {3: 5}, True,
                   trigger, next_uop, repeat, (1, 1))
        u.enable_input(InpSel.SRC_0, 1)
        u.enable_input(InpSel.SRC_0_HI, 2)
        u.enable_input(InpSel.SRC_1, 3)
        u.enable_input(InpSel.SRC_1_HI, 4)
        u.enable_input(InpSel.ONE_F32, 5)
        return u

    return fsm(mk_1x), fsm(mk_2x)


def _get_op():
    global _OP
    if _OP is not None:
        return _OP
    import concourse.dve_ops as dve_ops
    from concourse.dve_ops import OPS, DveOp
    from concourse.dve_spec import AluOp as SAluOp, Bin, Idx, Scan, Spec, Src0, Src1
    from concourse.dve_uop import DveOpSpec

    name = "MS2XU_ANT"
    for op in OPS:
        if op.name == name:
            _OP = op
            return op

    def _ref(in0, in1, s0, s1, imm2):
        x = in0.astype(np.float32)
        x = x.reshape(x.shape[0], -1, S)
        a = np.asarray(in1, np.float32).reshape(x.shape[0], -1, S)
        th = 2.0 * a[:, :, 0]
        mask = np.arange(S, dtype=np.float32)[None, None, :] < th[:, :, None]
        body = x * mask
        return np.cumsum(body, axis=2, dtype=np.float32).reshape(in0.shape[0], -1)

    masked = Bin(SAluOp.MULTIPLY, Bin(SAluOp.IS_LT, Idx, Src1), Src0)
    sc = object.__new__(Scan)
    object.__setattr__(sc, "op", SAluOp.ADD)
    object.__setattr__(sc, "expr", masked)
    object.__setattr__(sc, "init", None)
    object.__setattr__(sc, "_subdim_step", None)
    spec = Spec(body=sc, reference=_ref)

    class DveOpHand(DveOp):
        def compile(self, ver):
            return _HAND[(self.name, ver)]

    op = DveOpHand(name, spec, subdim=True, uops_sha={})
    OPS.append(op)
    row = dve_ops._CUSTOM_DVE_ROW_BASE + len(OPS) - 1
    dve_ops._SUB_OPCODE_FOR_NAME[name] = row
    dve_ops.CUSTOM_DVE_SPECS[name] = spec

    u1x, u2x = _build_programs()
    for ver in ("v3", "v4"):
        _HAND[(name, ver)] = DveOpSpec(
            name=name, uops=u1x, opcode=row, uops_2x=u2x, perf_max=1,
            rd1_en=True,
        )
    _OP = op
    return op


def _group_sizes(tiles, group):
    sizes = []
    rem = tiles
    for s_ in (2, 3, 5, 8, group // 2):
        if s_ >= 2 and rem - s_ >= group:
            sizes.append(s_)
            rem -= s_
    while rem > 0:
        s_ = min(group, rem)
        sizes.append(s_)
        rem -= s_
    return sizes


def build_nc(tiles=TILES, group=GROUP):
    import concourse.bacc as bacc
    import concourse.mybir as mybir
    from concourse import tile

    f32 = mybir.dt.float32
    f16 = mybir.dt.float16
    A_ = mybir.AluOpType
    AF = mybir.ActivationFunctionType
    OP = _get_op()

    if tiles < group:
        group = tiles
    sizes = _group_sizes(tiles, group)
    Tm = max(sizes)
    bpc = tiles * P
    SEED = 32 if S >= 64 else 2  # elements (8 pairs) seeded by DVE writes

    nc = bacc.Bacc("TRN2", target_bir_lowering=False, debug=False)
    inp = nc.dram_tensor("inp", [bpc, COLS], f32, kind="ExternalInput")
    out = nc.dram_tensor("out", [bpc, 3], f32, kind="ExternalOutput")

    vin = inp.ap().rearrange("(p t) c -> p t c", p=P)
    vout = out.ap().rearrange("(p t) k -> p t k", p=P)

    with tile.TileContext(nc) as tc:
        with (
            tc.tile_pool(name="raw", bufs=2) as rawp,
            tc.tile_pool(name="la", bufs=2) as lap,
            tc.tile_pool(name="idx", bufs=3) as idxp,
            tc.tile_pool(name="st", bufs=3) as stp,
            tc.tile_pool(name="jk", bufs=1) as jkp,
            tc.tile_pool(name="per", bufs=1) as perp,
        ):
            outlog = perp.tile([P, tiles, 3], f16)
            outf = perp.tile([P, tiles, 3], f32)
            junk = jkp.tile([P, Tm, S], f16)

            t0 = 0
            for T in sizes:
                rawf = rawp.tile([P, Tm, S], f32, tag="raw")
                raw = rawf[:, 0:T, :]
                nc.sync.dma_start(raw, vin[:, t0 : t0 + T, 0:S])
                icolsf = idxp.tile([P, Tm, 2], f32, tag="icols")
                if T < Tm:
                    nc.vector.memset(icolsf, 0.0)
                icols = icolsf[:, 0:T, :]
                nc.sync.dma_start(icols, vin[:, t0 : t0 + T, S:COLS])

                # seed 8 (A, B) pairs per page into the flat stream buffer:
                # stream i occupies pages [i*Tm, i*Tm+T); A at even offsets,
                # B = A - 0.5 at odd offsets. Seeds cover all Tm pages so the
                # doubling DMAs below never read uninitialized memory.
                stf = stp.tile([P, 3 * Tm, S], f16, tag="ast")
                nseed = SEED // 2
                bidb = icolsf[:, :, 0].unsqueeze(2).broadcast_to([P, Tm, nseed])
                mpb = icolsf[:, :, 1].unsqueeze(2).broadcast_to([P, Tm, nseed])
                for i, (srcb, add) in enumerate((
                    (bidb, 0.0),            # A = bid/2
                    (mpb, 0.5),             # A = (mp+1)/2
                    (mpb, 0.0),             # A = mp/2
                )):
                    pg = stf[:, i * Tm : (i + 1) * Tm, :]
                    evens = pg[:, :, 0 : SEED : 2]
                    odds = pg[:, :, 1 : SEED : 2]
                    nc.scalar.activation(
                        evens, srcb, AF.Copy, bias=add, scale=0.5
                    )
                    nc.scalar.activation(
                        odds, srcb, AF.Copy, bias=add - 0.5, scale=0.5
                    )

                # log-doubling replicate across all 3*Tm pages at once
                k = SEED
                while k < S:
                    n = min(k, S - k)
                    nc.sync.dma_start(
                        stf[:, :, k : k + n], stf[:, :, 0:n]
                    )
                    k += n

                la = lap.tile([P, Tm, S], f16, tag="la")
                lag = la[:, 0:T, :]
                nc.scalar.activation(
                    lag.rearrange("p t s -> p (t s)"),
                    raw.rearrange("p t s -> p (t s)"),
                    AF.Ln,
                )

                for i in range(3):
                    jg = junk[:, 0:T, :]
                    bi = nc.vector._custom_dve(
                        OP,
                        out=jg,
                        in0=lag,
                        in1=stf[:, i * Tm : i * Tm + T, :],
                        s0=0.5,
                    )
                    bi.ins.perf_max = 1
                    # page-end sums -> outlog[p, t, i]
                    nc.vector.tensor_copy(
                        outlog[:, t0 : t0 + T, i], jg[:, :, S - 1]
                    )
                t0 += T

            nc.scalar.activation(
                outf.rearrange("p t k -> p (t k)"),
                outlog.rearrange("p t k -> p (t k)"),
                AF.Exp,
            )
            nc.sync.dma_start(vout, outf)

    nc.compile()
    return nc


_NC_CACHE = {}


def _get_nc():
    key = (TILES, GROUP)
    if key not in _NC_CACHE:
        _NC_CACHE[key] = build_nc()
    return _NC_CACHE[key]


def kernel(inputs):
    global LAST_RESULTS
    x = np.ascontiguousarray(np.asarray(inputs), dtype=np.float32)
    assert x.shape == (BTOT, COLS), x.shape

    npad = BPC * NCORES - BTOT
    padrows = np.zeros((npad, COLS), dtype=np.float32)
    padrows[:, :S] = 1.0
    xp = np.concatenate([x, padrows], axis=0)
    shards = xp.reshape(NCORES, BPC, COLS)

    in_maps = [{"inp": np.ascontiguousarray(shards[c])} for c in range(NCORES)]

    nc = _get_nc()
    from concourse.bass_utils import run_bass_kernel_spmd

    r = run_bass_kernel_spmd(
        nc, in_maps, core_ids=list(range(NCORES)), trace=TRACE
    )
    LAST_RESULTS = r
    y = np.concatenate([r.results[c]["out"] for c in range(NCORES)], axis=0)
    return np.ascontiguousarray(y[:BTOT]).astype(np.float32)


# revision 18
# speedup vs baseline: 2.2705x; 1.9740x over previous
"""Trainium2 Bass kernel for BidPrefix, v5: fp16 2x_1P paged masked sums.

Reference semantics (per row b of inputs [B, 302]):
  rates = inputs[b, :300]; bid = int(inputs[b, 300]); mp = int(inputs[b, 301])
  cpz[k] = prod(rates[:k]) (cpz[0] = 1); out[b] = [cpz[bid], cpz[mp+1], cpz[mp]]

Log-space masked prefix sums (cpz[idx] = exp(sum_{j<idx} ln r_j)) with a
hand-authored 2-elements/cycle (2x_1P perf mode) fp16 custom DVE op,
validated on HW by probe2x.py:
  - ScalarE: la = ln(rates) -> fp16 [128, T, 300]
  - per output i (th = bid / mp+1 / mp): src1 = full-rate fp16 pair stream,
    page t = (A, B) = (th/2, (th-1)/2) repeated 150x. Built by seeding 8
    pairs with strided tensor_scalar writes, then 5 log-doubling SBUF->SBUF
    DMA copies over a flat [128, 3*Tm pages, k] layout (one DMA covers all
    3 streams).
  - custom op (3 uops: entry-reset / steady / reset at SUB_DIM_DONE): per
    cycle processes a packed fp16 pair: two pair-counter scan states c,
    m_lo = c < A, m_hi = c < B, S += m_lo*x_lo + m_hi*x_hi; counters and S
    reset each page, so the page-end out element is the per-row masked sum.
  - page-end sums -> one Exp pass (ScalarE) -> out.
"""

import sys

if "/opt/trn_rl_repo" not in sys.path:
    sys.path.insert(0, "/opt/trn_rl_repo")

import numpy as np

S = 300
COLS = 302
P = 128
NCORES = 8
TILES = 196
GROUP = 20
BPC = TILES * P
BTOT = 200000

TRACE = False
LAST_RESULTS = None

_OP = None
_HAND = {}


def _build_programs():
    from concourse.dve_uop import (
        AluInp, AluOp, DelayInp, InpSel, OutPath, OutSel, Trigger,
        UopConfig, UopDpConfig,
    )

    ENABLE = 1

    def mk_uop(stages, lanes_used, captures, out_both, trigger, next_uop,
               repeat, consume):
        u = UopConfig()
        dp = []
        for b in range(8):
            blk = UopDpConfig()
            blk.pass_through_delay(*lanes_used)
            if b in stages:
                op, s0, s1 = stages[b]
                blk.enable_alu(op, s0, s1)
            else:
                blk.pass_through_alu()
            if b in captures:
                blk.enable_delay_from_src(DelayInp.PREV_ALU_OUT, captures[b])
            dp.append(blk)
        u.datapath_config = dp
        u.out[OutPath.WR0_LO] = OutSel.ALU_OUT
        u.out_enable[OutPath.WR0_LO] = ENABLE
        if out_both:
            u.out[OutPath.WR0_HI] = OutSel.ALU_OUT
            u.out_enable[OutPath.WR0_HI] = ENABLE
        u.trigger = trigger
        u.next_uop = next_uop
        u.repeat_count = repeat
        u.require_inp0, u.require_inp1 = consume
        return u

    def fsm(mk_body):
        t_entry = (Trigger.SRC_TENSOR_DONE, Trigger.SUB_DIM_DONE, Trigger.COUNT)
        t_stdy = (Trigger.SRC_TENSOR_DONE, Trigger.SUB_DIM_DONE, Trigger.NONE)
        return [
            mk_body(True, t_entry, (0, 2, 1), 1),
            mk_body(False, t_stdy, (0, 2, 0), 0),
            mk_body(True, t_entry, (0, 2, 1), 1),
        ]

    PREV = AluInp.PREV_ALU_OUT
    CURR = AluInp.CURR_ALU_OUT
    D = [AluInp.PREV_DELAY_0, AluInp.PREV_DELAY_1, AluInp.PREV_DELAY_2,
         AluInp.PREV_DELAY_3, AluInp.PREV_DELAY_4, AluInp.PREV_DELAY_5]

    # 1x base: half-step counter vs alternating (A, B) stream elements.
    # lanes: L0=SRC_0, L1=SRC_1, L2=CONST_0(=0.5)
    def mk_1x(reset, trigger, next_uop, repeat):
        stages = {
            0: (AluOp.SUBTRACT, CURR, CURR) if reset else (AluOp.ADD, CURR, D[2]),
            1: (AluOp.IS_LT, PREV, D[1]),
            2: (AluOp.MULTIPLY, PREV, D[0]),
            3: (AluOp.BYPASS, PREV, PREV) if reset else (AluOp.ADD, CURR, PREV),
        }
        u = mk_uop(stages, (0, 1, 2), {}, False, trigger, next_uop, repeat,
                   (1, 1))
        u.enable_input(InpSel.SRC_0, 1)
        u.enable_input(InpSel.SRC_1, 2)
        u.enable_input(InpSel.CONST_0, 3)
        return u

    # 2x_1P (probe-validated): pair counter c in two scan states;
    # m_lo = c < A (SRC_1), m_hi = c < B (SRC_1_HI).
    # lanes: L0=SRC_0, L1=SRC_0_HI, L2=SRC_1, L3=SRC_1_HI, L4=ONE, L5=capture
    def mk_2x(reset, trigger, next_uop, repeat):
        ctr = (AluOp.SUBTRACT, CURR, CURR) if reset else (AluOp.ADD, CURR, D[4])
        stages = {
            0: ctr,                                # c
            1: (AluOp.IS_LT, PREV, D[2]),          # m_lo = c < A
            2: (AluOp.MULTIPLY, PREV, D[0]),       # v_lo = m_lo * x_lo
            3: ctr,                                # c (2nd scan) + capture v_lo
            4: (AluOp.IS_LT, PREV, D[3]),          # m_hi = c < B
            5: (AluOp.MULTIPLY, PREV, D[1]),       # v_hi
            6: (AluOp.ADD, PREV, D[5]),            # pair = v_hi + v_lo
            7: (AluOp.BYPASS, PREV, PREV) if reset else (AluOp.ADD, CURR, PREV),
        }
        u = mk_uop(stages, (0, 1, 2, 3, 4, 5), {3: 5}, True,
                   trigger, next_uop, repeat, (1, 1))
        u.enable_input(InpSel.SRC_0, 1)
        u.enable_input(InpSel.SRC_0_HI, 2)
        u.enable_input(InpSel.SRC_1, 3)
        u.enable_input(InpSel.SRC_1_HI, 4)
        u.enable_input(InpSel.ONE_F32, 5)
        return u

    return fsm(mk_1x), fsm(mk_2x)


def _get_op():
    global _OP
    if _OP is not None:
        return _OP
    import concourse.dve_ops as dve_ops
    from concourse.dve_ops import OPS, DveOp
    from concourse.dve_spec import AluOp as SAluOp, Bin, Idx, Scan, Spec, Src0, Src1
    from concourse.dve_uop import DveOpSpec

    name = "MS2XU_ANT"
    for op in OPS:
        if op.name == name:
            _OP = op
            return op

    def _ref(in0, in1, s0, s1, imm2):
        x = in0.astype(np.float32)
        x = x.reshape(x.shape[0], -1, S)
        a = np.asarray(in1, np.float32).reshape(x.shape[0], -1, S)
        th = 2.0 * a[:, :, 0]
        mask = np.arange(S, dtype=np.float32)[None, None, :] < th[:, :, None]
        body = x * mask
        return np.cumsum(body, axis=2, dtype=np.float32).reshape(in0.shape[0], -1)

    masked = Bin(SAluOp.MULTIPLY, Bin(SAluOp.IS_LT, Idx, Src1), Src0)
    sc = object.__new__(Scan)
    object.__setattr__(sc, "op", SAluOp.ADD)
    object.__setattr__(sc, "expr", masked)
    object.__setattr__(sc, "init", None)
    object.__setattr__(sc, "_subdim_step", None)
    spec = Spec(body=sc, reference=_ref)

    class DveOpHand(DveOp):
        def compile(self, ver):
            return _HAND[(self.name, ver)]

    op = DveOpHand(name, spec, subdim=True, uops_sha={})
    OPS.append(op)
    row = dve_ops._CUSTOM_DVE_ROW_BASE + len(OPS) - 1
    dve_ops._SUB_OPCODE_FOR_NAME[name] = row
    dve_ops.CUSTOM_DVE_SPECS[name] = spec

    u1x, u2x = _build_programs()
    for ver in ("v3", "v4"):
        _HAND[(name, ver)] = DveOpSpec(
            name=name, uops=u1x, opcode=row, uops_2x=u2x, perf_max=1,
            rd1_en=True,
        )
    _OP = op
    return op


def _group_sizes(tiles, group):
    sizes = []
    rem = tiles
    for s_ in (2, 3, 5, 8, group // 2):
        if s_ >= 2 and rem - s_ >= group:
            sizes.append(s_)
            rem -= s_
    while rem > 0:
        s_ = min(group, rem)
        sizes.append(s_)
        rem -= s_
    return sizes


def build_nc(tiles=TILES, group=GROUP):
    import concourse.bacc as bacc
    import concourse.mybir as mybir
    from concourse import tile

    f32 = mybir.dt.float32
    f16 = mybir.dt.float16
    A_ = mybir.AluOpType
    AF = mybir.ActivationFunctionType
    OP = _get_op()

    if tiles < group:
        group = tiles
    sizes = _group_sizes(tiles, group)
    Tm = max(sizes)
    bpc = tiles * P
    SEED = 16 if S >= 32 else 2  # elements (8 pairs) seeded by DVE writes

    nc = bacc.Bacc("TRN2", target_bir_lowering=False, debug=False)
    inp = nc.dram_tensor("inp", [bpc, COLS], f32, kind="ExternalInput")
    out = nc.dram_tensor("out", [bpc, 3], f32, kind="ExternalOutput")

    vin = inp.ap().rearrange("(p t) c -> p t c", p=P)
    vout = out.ap().rearrange("(p t) k -> p t k", p=P)

    with tile.TileContext(nc) as tc:
        with (
            tc.tile_pool(name="raw", bufs=2) as rawp,
            tc.tile_pool(name="la", bufs=2) as lap,
            tc.tile_pool(name="idx", bufs=3) as idxp,
            tc.tile_pool(name="st", bufs=2) as stp,
            tc.tile_pool(name="jk", bufs=1) as jkp,
            tc.tile_pool(name="per", bufs=1) as perp,
        ):
            outlog = perp.tile([P, tiles, 3], f16)
            outf = perp.tile([P, tiles, 3], f32)
            junk = jkp.tile([P, Tm, S], f16)

            t0 = 0
            for T in sizes:
                rawf = rawp.tile([P, Tm, S], f32, tag="raw")
                raw = rawf[:, 0:T, :]
                nc.sync.dma_start(raw, vin[:, t0 : t0 + T, 0:S])
                icolsf = idxp.tile([P, Tm, 2], f32, tag="icols")
                if T < Tm:
                    nc.vector.memset(icolsf, 0.0)
                icols = icolsf[:, 0:T, :]
                nc.sync.dma_start(icols, vin[:, t0 : t0 + T, S:COLS])

                # seed 8 (A, B) pairs per page into the flat stream buffer:
                # stream i occupies pages [i*Tm, i*Tm+T); A at even offsets,
                # B = A - 0.5 at odd offsets. Seeds cover all Tm pages so the
                # doubling DMAs below never read uninitialized memory.
                stf = stp.tile([P, 3 * Tm, S], f16, tag="ast")
                nh = S // 2
                bidb = icolsf[:, :, 0].unsqueeze(2).broadcast_to([P, Tm, nh])
                mpb = icolsf[:, :, 1].unsqueeze(2).broadcast_to([P, Tm, nh])
                for i, (srcb, add) in enumerate((
                    (bidb, 0.0),            # A = bid/2
                    (mpb, 0.5),             # A = (mp+1)/2
                    (mpb, 0.0),             # A = mp/2
                )):
                    pg = stf[:, i * Tm : (i + 1) * Tm, :]
                    evens = pg[:, :, 0:S:2]
                    odds = pg[:, :, 1:S:2]
                    if i < 2:
                        nc.scalar.activation(
                            evens, srcb, AF.Copy, bias=add, scale=0.5
                        )
                        nc.scalar.activation(
                            odds, srcb, AF.Copy, bias=add - 0.5, scale=0.5
                        )
                    else:
                        nc.vector.tensor_scalar(
                            evens, srcb, 0.5, add, A_.mult, A_.add
                        )
                        nc.vector.tensor_scalar(
                            odds, srcb, 0.5, add - 0.5, A_.mult, A_.add
                        )

                la = lap.tile([P, Tm, S], f16, tag="la")
                lag = la[:, 0:T, :]
                nc.scalar.activation(
                    lag.rearrange("p t s -> p (t s)"),
                    raw.rearrange("p t s -> p (t s)"),
                    AF.Ln,
                )

                for i in range(3):
                    jg = junk[:, 0:T, :]
                    bi = nc.vector._custom_dve(
                        OP,
                        out=jg,
                        in0=lag,
                        in1=stf[:, i * Tm : i * Tm + T, :],
                        s0=0.5,
                    )
                    bi.ins.perf_max = 1
                    # page-end sums -> outlog[p, t, i]
                    nc.vector.tensor_copy(
                        outlog[:, t0 : t0 + T, i], jg[:, :, S - 1]
                    )
                t0 += T

            nc.scalar.activation(
                outf.rearrange("p t k -> p (t k)"),
                outlog.rearrange("p t k -> p (t k)"),
                AF.Exp,
            )
            nc.sync.dma_start(vout, outf)

    nc.compile()
    return nc


_NC_CACHE = {}


def _get_nc():
    key = (TILES, GROUP)
    if key not in _NC_CACHE:
        _NC_CACHE[key] = build_nc()
    return _NC_CACHE[key]


def kernel(inputs):
    global LAST_RESULTS
    x = np.ascontiguousarray(np.asarray(inputs), dtype=np.float32)
    assert x.shape == (BTOT, COLS), x.shape

    npad = BPC * NCORES - BTOT
    padrows = np.zeros((npad, COLS), dtype=np.float32)
    padrows[:, :S] = 1.0
    xp = np.concatenate([x, padrows], axis=0)
    shards = xp.reshape(NCORES, BPC, COLS)

    in_maps = [{"inp": np.ascontiguousarray(shards[c])} for c in range(NCORES)]

    nc = _get_nc()
    from concourse.bass_utils import run_bass_kernel_spmd

    r = run_bass_kernel_spmd(
        nc, in_maps, core_ids=list(range(NCORES)), trace=TRACE
    )
    LAST_RESULTS = r
    y = np.concatenate([r.results[c]["out"] for c in range(NCORES)], axis=0)
    return np.ascontiguousarray(y[:BTOT]).astype(np.float32)
